# revision 29
# baseline (speedup 1.0000x reference)
"""Trainium2 Bass kernel for the FGN layer.

out[b,o] = (x @ W.T + bias) * exp(-||x_b - c_o||^2 / sig_o^2)

Strategy: data-parallel over batch (8 cores x 1024 rows). Per core, two
GEMMs with out-features on PSUM partitions:
  l[o,b]  = sum_k W.T[k,o] * x.T[k,b]        in fp32r (precision-critical)
  m2[o,b] = sum_k (-2*C).T[k,o] * x.T[k,b]   in bf16 (g is insensitive:
            d2 error ~0.5 on a ~4096 scale, and sig^2 ~ 4e6)
Epilogue per 128-row o-tile:
  s = m2 + x_sq                      (DVE)
  g = exp(s*(-1/sig^2) - c_sq/sig^2) (ACT, fused scale+bias)
  out = (l + bias) * g               (DVE scalar_tensor_tensor)

Host preps SBUF-image layouts (each weight slab is stored exactly as its
SBUF tile image so DMAs move long contiguous lines), per-row reductions,
bf16 cast of -2*centers, and the final gather/transpose. DMA dispatch is
spread across the sync (x + W), scalar (C + consts) and gpsimd (stores)
queues to keep descriptor generation off the critical path.
"""
import numpy as np
import ml_dtypes
from contextlib import ExitStack

import concourse.bass as bass
import concourse.tile as tile
from concourse import bacc, mybir
from concourse.bass_utils import run_bass_kernel_spmd

F32 = mybir.dt.float32
F32R = mybir.dt.float32r
BF16 = mybir.dt.bfloat16
FP8 = mybir.dt.float8e4

B, IN, OUT = 8192, 2048, 2048
NCORES = 8
BS = B // NCORES       # 1024 batch rows per core
KC = IN // 128         # 16 contraction chunks
OT = OUT // 128        # 16 output tiles
MOV = 512              # moving free dim per matmul
BH = BS // MOV         # 2 batch halves
WQ = 4                 # w-slab DMA/rounding quarters
CD = 2                 # c-slab DMA splits

_NC_CACHE = {}


def _build_nc():
    if "nc" in _NC_CACHE:
        return _NC_CACHE["nc"]
    nc = bacc.Bacc("TRN2", target_bir_lowering=False, debug=False)

    # All 2D weight inputs are SBUF-image layouts: [.., 128, cols]
    xt_d = nc.dram_tensor("xt", [KC, 128, BS], F32, kind="ExternalInput").ap()
    wt_d = nc.dram_tensor("wt", [OT, 128, KC * 128], F32,
                          kind="ExternalInput").ap()
    ct_d = nc.dram_tensor("ct", [OT, 128, KC * 128], FP8,
                          kind="ExternalInput").ap()
    xsq_d = nc.dram_tensor("xsq", [1, BS], F32, kind="ExternalInput").ap()
    vb_d = nc.dram_tensor("vb", [128, OT], F32, kind="ExternalInput").ap()
    vs_d = nc.dram_tensor("vs", [128, OT], F32, kind="ExternalInput").ap()
    va_d = nc.dram_tensor("va", [128, OT], F32, kind="ExternalInput").ap()
    out_d = nc.dram_tensor("out", [OUT, BS], F32, kind="ExternalOutput").ap()

    WCOL = KC * 128            # 2048 slab columns
    WQC = WCOL // WQ           # 512 cols per w quarter
    CDC = WCOL // CD           # 1024 cols per c split

    with tile.TileContext(nc) as tc:
        with ExitStack() as ctx:
            const = ctx.enter_context(tc.tile_pool(name="const", bufs=1))
            stage = ctx.enter_context(tc.tile_pool(name="stage", bufs=3))
            wcst = ctx.enter_context(tc.tile_pool(name="wcst", bufs=3))
            wcr = ctx.enter_context(tc.tile_pool(name="wcr", bufs=3))
            cbf = ctx.enter_context(tc.tile_pool(name="cbf", bufs=3))
            temps = ctx.enter_context(tc.tile_pool(name="temps", bufs=2))
            outp = ctx.enter_context(tc.tile_pool(name="outp", bufs=3))
            psum = ctx.enter_context(tc.tile_pool(name="psum", bufs=4, space="PSUM"))

            x_res = const.tile([128, KC * BS], F32R)
            x_f8 = const.tile([128, KC * BS], FP8)
            x_f8_3d = x_f8[:].rearrange("p (c b) -> p c b", b=BS)



            def load_x_chunk(k):
                xs = stage.tile([128, BS], F32, tag="xs")
                eng = nc.sync if k % 2 == 0 else nc.scalar
                for q in range(4):
                    qs = slice(q * (BS // 4), (q + 1) * (BS // 4))
                    eng.dma_start(xs[:, qs], xt_d[k, :, qs])
                # separate rounding per half so matmul h only waits on half h
                for h in range(BH):
                    sl = slice(k * BS + h * MOV, k * BS + (h + 1) * MOV)
                    hs = slice(h * MOV, (h + 1) * MOV)
                    nc.vector.tensor_copy(x_res[:, sl], xs[:, hs])
                    nc.vector.tensor_copy(x_f8[:, sl], xs[:, hs])

            w_tiles, c_tiles = {}, {}

            def load_slab(t, splits=1):
                w_s = wcst.tile([128, WCOL], F32, tag="w_s")
                wn = WQ * splits
                wc = WCOL // wn
                for q in range(wn):
                    eng = nc.sync if q % 2 == 0 else nc.scalar
                    eng.dma_start(w_s[:, q * wc:(q + 1) * wc],
                                  wt_d[t, :, q * wc:(q + 1) * wc])
                c_r = cbf.tile([128, WCOL], FP8, tag="c_r")
                cn = CD * splits
                cc = WCOL // cn
                for d in range(cn):
                    nc.scalar.dma_start(c_r[:, d * cc:(d + 1) * cc],
                                        ct_d[t, :, d * cc:(d + 1) * cc])
                w_r = wcr.tile([128, WCOL], F32R, tag="w_r")
                for q in range(WQ):
                    sl = slice(q * WQC, (q + 1) * WQC)
                    nc.vector.tensor_copy(w_r[:, sl], w_s[:, sl])
                w_tiles[t] = w_r
                c_tiles[t] = c_r

            # Prologue: first x chunk + first slab before the x bulk.
            load_x_chunk(0)
            load_slab(0, splits=2)
            for k in range(1, KC):
                load_x_chunk(k)

            # Epilogue constants (first needed ~20us in)
            xsq_t = const.tile([128, BS], F32)
            for q in range(4):
                nc.scalar.dma_start(xsq_t[q * 32:(q + 1) * 32, :],
                                    xsq_d.to_broadcast((32, BS)))
            vb_t = const.tile([128, OT], F32)
            nc.scalar.dma_start(vb_t[:], vb_d[:, :])
            vs_t = const.tile([128, OT], F32)
            nc.scalar.dma_start(vs_t[:], vs_d[:, :])
            va_t = const.tile([128, OT], F32)
            nc.scalar.dma_start(va_t[:], va_d[:, :])

            for t in range(OT):
                if t not in w_tiles:
                    load_slab(t)
                w_r, c_r = w_tiles.pop(t), c_tiles.pop(t)

                l_ps = psum.tile([128, BS], F32, tag="ps")
                m_ps = psum.tile([128, BS], F32, tag="ps")
                c_3d = c_r[:].rearrange("p (c j) -> p c j", j=128)
                for k in range(KC):
                    st, sp = (k == 0), (k == KC - 1)
                    wk = w_r[:, k * 128:(k + 1) * 128]
                    for h in range(BH):
                        mv = x_res[:, k * BS + h * MOV: k * BS + (h + 1) * MOV]
                        nc.tensor.matmul(l_ps[:, h * MOV:(h + 1) * MOV],
                                         wk, mv, start=st, stop=sp)

                for kp in range(KC // 2):
                    # fp8 DoubleRow: one matmul contracts chunks 2kp, 2kp+1
                    ck = c_3d[:, 2 * kp:2 * kp + 2, :]
                    for h in range(BH):
                        mv = x_f8_3d[:, 2 * kp:2 * kp + 2,
                                     h * MOV:(h + 1) * MOV]
                        nc.tensor.matmul(
                            m_ps[:, h * MOV:(h + 1) * MOV], ck, mv,
                            start=(kp == 0), stop=(kp == KC // 2 - 1),
                            perf_mode=mybir.MatmulPerfMode.DoubleRow)

                s_t = temps.tile([128, BS], F32, tag="s")
                nc.vector.tensor_tensor(s_t[:], m_ps[:], xsq_t[:],
                                        op=mybir.AluOpType.add)
                g_t = temps.tile([128, BS], F32, tag="g")
                nc.scalar.activation(g_t[:], s_t[:],
                                     mybir.ActivationFunctionType.Exp,
                                     bias=va_t[:, t:t + 1],
                                     scale=vs_t[:, t:t + 1])
                o_t = outp.tile([128, BS], F32)
                nc.vector.scalar_tensor_tensor(o_t[:], l_ps[:], vb_t[:, t:t + 1],
                                               g_t[:],
                                               op0=mybir.AluOpType.add,
                                               op1=mybir.AluOpType.mult)
                for q in range(4):
                    hs = slice(q * (BS // 4), (q + 1) * (BS // 4))
                    eng = nc.gpsimd if q % 2 == 0 else nc.sync
                    eng.dma_start(out_d[t * 128:(t + 1) * 128, hs],
                                  o_t[:, hs])

    nc.finalize()
    _NC_CACHE["nc"] = nc
    return nc


def _prep_inputs(x, weights, centers, sigs):
    x = np.asarray(x, np.float32)
    weights = np.asarray(weights, np.float32)
    centers = np.asarray(centers, np.float32)
    sigs = np.asarray(sigs, np.float32)

    # SBUF-image slab layout: img[t, p, k*128+j] = M[t*128+j, k*128+p]
    def slab_image(m):
        m4 = m.reshape(OT, 128, KC, 128)          # [t, j, k, p]
        return np.ascontiguousarray(
            m4.transpose(0, 3, 2, 1).reshape(OT, 128, KC * 128))

    wt = slab_image(weights)
    ct = slab_image((-2.0 * centers)).astype(ml_dtypes.float8_e4m3)

    w64 = weights.astype(np.float64)
    c64 = centers.astype(np.float64)
    biases = -(w64 * c64).sum(axis=1)
    c_sq = (c64 * c64).sum(axis=1)
    inv_sig2 = 1.0 / (sigs.astype(np.float64) ** 2)

    def ovec(v):
        return np.ascontiguousarray(
            v.astype(np.float32).reshape(OT, 128).T)

    vb = ovec(biases)
    vs = ovec(-inv_sig2)
    va = ovec(-c_sq * inv_sig2)

    in_maps = []
    for c in range(NCORES):
        xs = x[c * BS:(c + 1) * BS]
        in_maps.append({
            "xt": np.ascontiguousarray(xs.T).reshape(KC, 128, BS),
            "wt": wt,
            "ct": ct,
            "xsq": (xs.astype(np.float64) ** 2).sum(axis=1)
                   .astype(np.float32).reshape(1, BS),
            "vb": vb,
            "vs": vs,
            "va": va,
        })
    return in_maps


def _run(in_maps, trace=False):
    nc = _build_nc()
    return run_bass_kernel_spmd(nc, in_maps, core_ids=list(range(NCORES)),
                                trace=trace)


def kernel(x, weights, centers, sigs):
    in_maps = _prep_inputs(x, weights, centers, sigs)
    res = _run(in_maps, trace=False)
    out = np.empty((B, OUT), np.float32)
    for c in range(NCORES):
        out[c * BS:(c + 1) * BS, :] = res.results[c]["out"].T
    return out
